# revision 1
# baseline (speedup 1.0000x reference)
"""MoE (top-2 of 8 experts, SwiGLU) Trainium2 kernel.

Strategy (expert-parallel over 8 NeuronCores):
  * Host: router GEMM + top-2 + sigmoid gates in numpy (selection verified to
    match the jax fp32 reference on these inputs), then gather each expert's
    tokens into a transposed, capacity-padded buffer xT_e [H, C]. One expert
    per core.
  * Device (SPMD, per core): two phases.
      Phase 1: h = silu(x @ Wg) * (x @ Wu), Wg/Wu SBUF-resident, h spilled
               to a DRAM scratch buffer (layout [I, C]).
      Phase 2: yT = (h @ Wd) * gate, Wd SBUF-resident, gate applied during
               PSUM eviction (out column t scaled by gate[t]).
    Matmuls run in float32r — IEEE fp32 layout with the mantissa rounded to
    11 bits (low 12 bits zero), which streams at full PE rate (1 cycle/row
    for moving dim >= 256) vs 4 cycles/row for fp32. Inputs are pre-rounded
    on the host (round-to-nearest-even bit trick); the h intermediate is
    rounded on-chip for free by giving the DVE multiply an fp32r output.
    Tokens are the moving dimension (512 wide), weights the 128x128
    stationary operand.
  * Host: out[idx_e] += yT_e[:, :n_e].T  (indices within one expert are
    unique, so fancy-index += is safe).
"""

import os
import numpy as np

T, H, I, E, TOPK = 8192, 1024, 2048, 8, 2
NCORES = 8
PB = 128

_compiled = {}
last_results = None  # BassKernelResults of the most recent run (for test harness)


def round_fp32r(a):
    """Round fp32 array to fp32r (11-bit mantissa, RNE), keeping fp32 layout."""
    u = np.ascontiguousarray(a, dtype=np.float32).view(np.uint32)
    r = (u + np.uint32(0x7FF) + ((u >> np.uint32(12)) & np.uint32(1))) \
        & np.uint32(0xFFFFF000)
    return r.view(np.float32)


def _tsegs(C):
    """Split C into segments of width 256..512 (fp32r full rate needs >=256)."""
    widths = []
    rem = C
    while rem >= 768:
        widths.append(512)
        rem -= 512
    if rem <= 512:
        widths.append(rem)
    else:
        widths.append(rem - 256)
        widths.append(256)
    segs = []
    t0 = 0
    for tb in widths:
        segs.append((t0, tb))
        t0 += tb
    return segs


def _build(C):
    import concourse.bacc as bacc
    import concourse.mybir as mybir
    import concourse.tile as tile

    fp32 = mybir.dt.float32
    fp32r = mybir.dt.float32r
    AF = mybir.ActivationFunctionType

    KB = H // PB   # 8 contraction blocks over H
    IB = I // PB   # 16 blocks over I
    HB = H // PB   # 8 output blocks over H

    nc = bacc.Bacc("TRN2", target_bir_lowering=False, debug=False,
                   num_devices=NCORES)
    xT = nc.dram_tensor("xT", [H, C], fp32r, kind="ExternalInput").ap()
    gm = nc.dram_tensor("gm", [PB, C], fp32, kind="ExternalInput").ap()
    Wg = nc.dram_tensor("Wg", [H, I], fp32r, kind="ExternalInput").ap()
    Wu = nc.dram_tensor("Wu", [H, I], fp32r, kind="ExternalInput").ap()
    Wd = nc.dram_tensor("Wd", [I, H], fp32r, kind="ExternalInput").ap()
    yT = nc.dram_tensor("yT", [H, C], fp32, kind="ExternalOutput").ap()
    hsp = nc.dram_tensor("hsp", [I, C], fp32r, kind="Internal").ap()

    segs = _tsegs(C)

    QW = 512           # weight-column quarter width
    NQ = I // QW       # 4 quarters
    IPQ = QW // PB     # 4 i-blocks per quarter

    # Split token segments into pair-groups so per-group x tiles fit in SBUF
    # while the quarter loop runs outermost (weights stream exactly once).
    halves = [segs[i:i + 2] for i in range(0, len(segs), 2)]
    N_EARLY = 4 if len(segs) > 1 else 0   # Wd tiles preloaded during phase 1

    from contextlib import ExitStack
    with tile.TileContext(nc) as tc, ExitStack() as _stack:
        wde = _stack.enter_context(tc.tile_pool(name="wde", bufs=1, side="right"))
        # Phase 1: h = silu(x@Wg) * (x@Wu) -> DRAM spill (fp32r)
        with tc.tile_pool(name="w1", bufs=1) as w1, \
             tc.tile_pool(name="xp", bufs=1) as xp, \
             tc.tile_pool(name="ev1", bufs=2) as ev1, \
             tc.tile_pool(name="ps1", bufs=3, space="PSUM") as ps1:
            wg_s = [[None] * NQ for _ in range(KB)]
            wu_s = [[None] * NQ for _ in range(KB)]

            def load_xt(si, t0, tb):
                tiles = []
                for k in range(KB):
                    xtk = xp.tile([PB, tb], fp32r, tag=f"xt{k}_{si}",
                                  name=f"xt{k}_{si}")
                    nc.sync.dma_start(
                        out=xtk[:], in_=xT[k * PB:(k + 1) * PB, t0:t0 + tb])
                    tiles.append(xtk)
                return tiles

            # q0 weight tiles live in their own pool, closed after their last
            # use so the freed SBUF can preload Wd tiles before phase 2.
            w1q0_stack = ExitStack()
            w1q0 = w1q0_stack.enter_context(tc.tile_pool(name="w1q0", bufs=1))

            # Interleave the first x tiles with the q0 gate weights so the
            # first matmul can issue after ~0.5MB of DMA.
            xt_half = []
            t0_0, tb_0 = halves[0][0]
            first_xt = []
            for k in range(KB):
                xtk = xp.tile([PB, tb_0], fp32r, tag=f"xt{k}_0", name=f"xt{k}_0")
                nc.sync.dma_start(
                    out=xtk[:], in_=xT[k * PB:(k + 1) * PB, t0_0:t0_0 + tb_0])
                first_xt.append(xtk)
                wgk = w1q0.tile([PB, QW], fp32r, name=f"wg{k}_0")
                nc.sync.dma_start(out=wgk[:], in_=Wg[k * PB:(k + 1) * PB, 0:QW])
                wg_s[k][0] = wgk
            for k in range(KB):
                wuk = w1q0.tile([PB, QW], fp32r, name=f"wu{k}_0")
                nc.sync.dma_start(out=wuk[:], in_=Wu[k * PB:(k + 1) * PB, 0:QW])
                wu_s[k][0] = wuk
            # Rest of half-0 x tiles, then remaining weight quarters.
            xt_half.append([first_xt] + [load_xt(si, t0, tb)
                                         for si, (t0, tb)
                                         in enumerate(halves[0][1:], start=1)])
            for q in range(1, NQ):
                for k in range(KB):
                    wgk = w1.tile([PB, QW], fp32r, name=f"wg{k}_{q}")
                    nc.sync.dma_start(
                        out=wgk[:], in_=Wg[k * PB:(k + 1) * PB, q * QW:(q + 1) * QW])
                    wg_s[k][q] = wgk
                for k in range(KB):
                    wuk = w1.tile([PB, QW], fp32r, name=f"wu{k}_{q}")
                    nc.sync.dma_start(
                        out=wuk[:], in_=Wu[k * PB:(k + 1) * PB, q * QW:(q + 1) * QW])
                    wu_s[k][q] = wuk
            # Preload the first Wd tiles during phase 1 (disjoint SBUF).
            wd_early = []
            for ib in range(N_EARLY):
                wdk = wde.tile([PB, H], fp32r, name=f"wde{ib}")
                nc.sync.dma_start(out=wdk[:], in_=Wd[ib * PB:(ib + 1) * PB, :])
                wd_early.append(wdk)

            for hi, half in enumerate(halves):
                if hi > 0:
                    xt_half.append([load_xt(si, t0, tb)
                                    for si, (t0, tb) in enumerate(half)])
                for q in range(NQ):
                    if hi == len(halves) - 1 and q == 1:
                        # q0 is dead everywhere: release its SBUF and use it
                        # to preload 8 more Wd tiles during the phase-1 tail.
                        w1q0_stack.close()
                        w2a = _stack.enter_context(
                            tc.tile_pool(name="w2a", bufs=1, side="right"))
                        for ib in range(N_EARLY, min(IB, N_EARLY + 8)):
                            wdk = w2a.tile([PB, H], fp32r, name=f"wda{ib}")
                            nc.sync.dma_start(
                                out=wdk[:], in_=Wd[ib * PB:(ib + 1) * PB, :])
                            wd_early.append(wdk)
                    for si, (t0, tb) in enumerate(half):
                        xt = xt_half[hi][si]
                        for ii in range(IPQ):
                            ib = q * IPQ + ii
                            qc = ii * PB
                            pg = ps1.tile([PB, tb], fp32, tag="pg", name="pg")
                            pu = ps1.tile([PB, tb], fp32, tag="pu", name="pu")
                            for k in range(KB):
                                nc.tensor.matmul(
                                    pg[:],
                                    wg_s[k][q][:, qc:qc + PB],
                                    xt[k][:],
                                    start=(k == 0), stop=(k == KB - 1))
                            for k in range(KB):
                                nc.tensor.matmul(
                                    pu[:],
                                    wu_s[k][q][:, qc:qc + PB],
                                    xt[k][:],
                                    start=(k == 0), stop=(k == KB - 1))
                            sg = ev1.tile([PB, tb], fp32, tag="sg", name="sg")
                            nc.scalar.activation(sg[:], pg[:], AF.Sigmoid)
                            sx = ev1.tile([PB, tb], fp32, tag="sx", name="sx")
                            nc.vector.tensor_mul(sx[:], sg[:], pg[:])
                            hh = ev1.tile([PB, tb], fp32r, tag="hh", name="hh")
                            nc.vector.tensor_mul(hh[:], sx[:], pu[:])
                            nc.gpsimd.dma_start(
                                out=hsp[ib * PB:(ib + 1) * PB, t0:t0 + tb],
                                in_=hh[:])

        # Phase 2: yT = (h @ Wd) * gate.  ib-outer: all 8 output blocks
        # accumulate in 8 PSUM banks so compute starts after wd0+ht0 land.
        with tc.tile_pool(name="w2", bufs=1) as w2, \
             tc.tile_pool(name="hl", bufs=3) as hl, \
             tc.tile_pool(name="ev2", bufs=8) as ev2, \
             tc.tile_pool(name="ps2", bufs=1, space="PSUM") as ps2:
            def load_ht(t0, tb, ib):
                htk = hl.tile([PB, tb], fp32r, tag=f"ht{ib}", name=f"ht{ib}")
                nc.sync.dma_start(
                    out=htk[:], in_=hsp[ib * PB:(ib + 1) * PB, t0:t0 + tb])
                return htk

            # Interleave remaining wd tiles with seg-0 h tiles in need-order.
            wd_s = list(wd_early)
            ht_next = []   # seg0 tiles
            for ib in range(IB):
                if ib >= len(wd_early):
                    wdk = w2.tile([PB, H], fp32r, name=f"wd{ib}")
                    nc.sync.dma_start(out=wdk[:], in_=Wd[ib * PB:(ib + 1) * PB, :])
                    wd_s.append(wdk)
                ht_next.append(load_ht(segs[0][0], segs[0][1], ib))
            gt = w2.tile([PB, C], fp32, name="gt")
            nc.sync.dma_start(out=gt[:], in_=gm[:])
            for si, (t0, tb) in enumerate(segs):
                ht = ht_next
                # queue the next segment's h tiles
                if si + 1 < len(segs):
                    nt0, ntb = segs[si + 1]
                    ht_next = [load_ht(nt0, ntb, ib) for ib in range(IB)]
                py = [ps2.tile([PB, tb], fp32, tag=f"py{hb}", name=f"py{hb}")
                      for hb in range(HB)]
                for ib in range(IB):
                    last = ib == IB - 1
                    for hb in range(HB):
                        nc.tensor.matmul(
                            py[hb][:],
                            wd_s[ib][:, hb * PB:(hb + 1) * PB],
                            ht[ib][:],
                            start=(ib == 0), stop=last)
                        if last:
                            # evict as soon as this output block finishes;
                            # the final segment flushes on the idle HWDGE
                            # queue (all loads are done by then).
                            yt = ev2.tile([PB, tb], fp32, tag="yt", name="yt")
                            nc.vector.tensor_mul(yt[:], py[hb][:],
                                                 gt[:, t0:t0 + tb])
                            eng = (nc.sync if si == len(segs) - 1
                                   else nc.gpsimd)
                            eng.dma_start(
                                out=yT[hb * PB:(hb + 1) * PB, t0:t0 + tb],
                                in_=yt[:])
    nc.compile()
    return nc


def _route(x, Wr, br):
    """Replicate the reference's fp32 router bit-compatibly on host."""
    logits = x @ Wr + br                       # fp32 GEMM
    order = np.argsort(-logits, axis=1, kind="stable")  # ties -> lowest index
    topk_idx = order[:, :TOPK]
    topk_vals = np.take_along_axis(logits, topk_idx, axis=1)
    g = 1.0 / (1.0 + np.exp(-topk_vals.astype(np.float32)))
    g = g / (np.sum(g, axis=-1, keepdims=True) + 1e-10)
    return topk_idx, g.astype(np.float32)


def kernel(x, Wr, br, Wg, Wu, Wd):
    global last_results
    from concourse.bass_utils import run_bass_kernel_spmd

    x = np.asarray(x, dtype=np.float32)
    Wr = np.asarray(Wr, dtype=np.float32)
    br = np.asarray(br, dtype=np.float32)
    Wg = np.asarray(Wg, dtype=np.float32)
    Wu = np.asarray(Wu, dtype=np.float32)
    Wd = np.asarray(Wd, dtype=np.float32)

    topk_idx, g = _route(x, Wr, br)

    # Per-expert token lists
    idx_lists = []
    gate_lists = []
    for e in range(E):
        mask = topk_idx == e                    # [T, K]
        tok = np.nonzero(mask.any(axis=1))[0]
        # gate value for expert e per selected token (slot 0 or slot 1)
        gsel = np.where(mask[tok, 0], g[tok, 0], g[tok, 1]).astype(np.float32)
        idx_lists.append(tok.astype(np.int64))
        gate_lists.append(gsel)

    counts = [len(ix) for ix in idx_lists]
    C = max(512, max(counts))

    key = C
    if key not in _compiled:
        _compiled[key] = _build(C)
    nc = _compiled[key]

    xTf = round_fp32r(np.ascontiguousarray(x.T))   # [H, T], pre-rounded
    in_maps = []
    for e in range(E):
        n = counts[e]
        xTe = np.zeros((H, C), dtype=np.float32)
        xTe[:, :n] = xTf[:, idx_lists[e]]
        gme = np.zeros((PB, C), dtype=np.float32)
        gme[:, :n] = gate_lists[e][None, :]
        in_maps.append({
            "xT": xTe,
            "gm": gme,
            "Wg": round_fp32r(Wg[e]),
            "Wu": round_fp32r(Wu[e]),
            "Wd": round_fp32r(Wd[e]),
        })

    trace = bool(int(os.environ.get("MOE_TRACE", "0")))
    trace_cores = (list(range(NCORES))
                   if os.environ.get("MOE_TRACE_ALL") else None)
    last_results = run_bass_kernel_spmd(
        nc, in_maps, core_ids=list(range(NCORES)), trace=trace,
        trace_cores=trace_cores)

    out = np.zeros((T, H), dtype=np.float32)
    for e in range(E):
        n = counts[e]
        yTe = last_results.results[e]["yT"]
        out[idx_lists[e]] += yTe[:, :n].T
    return out



# revision 2
# speedup vs baseline: 1.0280x; 1.0280x over previous
"""MoE (top-2 of 8 experts, SwiGLU) Trainium2 kernel.

Strategy (expert-parallel over 8 NeuronCores):
  * Host: router GEMM + top-2 + sigmoid gates in numpy (selection matches the
    jax fp32 reference on these inputs), then gather each expert's tokens into
    a transposed, capacity-padded bf16 buffer xT_e [H, C]. One expert per core.
  * Device (SPMD, per core): fully fused single pass in bf16.
    All weights (Wg, Wu, Wd ~ 12.6 MB bf16) stay SBUF-resident; the h
    intermediate never leaves SBUF (no DRAM spill).  Per 512-token segment:
      A: for each of 16 I-blocks: pg = x@Wg, pu = x@Wu (PSUM, 8 k-matmuls
         each), then h = silu(pg)*pu via Scalar(Silu) + one DVE mul with a
         bf16 output into SBUF.
      B: yT = (h @ Wd) * gate in two half-passes of 4 output blocks
         (4 PSUM banks each; A double-buffers the other 4), gates applied
         during PSUM eviction.
    bf16 matters twice: LDWEIGHTS of a 128x128 bf16 stationary (~0.1us with
    FWL) hides fully under the 213ns/512-row matmul stream, whereas fp32r
    stationary loads (~224ns) gate the pipe; and all weight/x DMA halves.
  * Host: out[idx_e] += yT_e[:, :n_e].T  (indices within one expert are
    unique, so fancy-index += is safe).
"""

import os
import numpy as np
import ml_dtypes

T, H, I, E, TOPK = 8192, 1024, 2048, 8, 2
NCORES = 8
PB = 128
KB = H // PB     # 8 contraction blocks over H
IB = I // PB     # 16 blocks over I
HB = H // PB     # 8 output blocks over H
WCOL = 256       # Wg/Wu load-tile column width (2 I-blocks per tile)
NWC = I // WCOL  # 8 column tiles

_compiled = {}
last_results = None  # BassKernelResults of the most recent run (for test harness)


def _tsegs(C):
    """Split C into 512-wide segments plus a remainder segment."""
    segs = []
    t0 = 0
    while C - t0 >= 512:
        segs.append((t0, 512))
        t0 += 512
    if C - t0:
        segs.append((t0, C - t0))
    return segs


def _build(C):
    import concourse.bacc as bacc
    import concourse.mybir as mybir
    import concourse.tile as tile

    fp32 = mybir.dt.float32
    bf16 = mybir.dt.bfloat16
    AF = mybir.ActivationFunctionType

    nc = bacc.Bacc("TRN2", target_bir_lowering=False, debug=False,
                   num_devices=NCORES)
    xT = nc.dram_tensor("xT", [H, C], bf16, kind="ExternalInput").ap()
    gm = nc.dram_tensor("gm", [PB, C], fp32, kind="ExternalInput").ap()
    Wg = nc.dram_tensor("Wg", [H, I], bf16, kind="ExternalInput").ap()
    Wu = nc.dram_tensor("Wu", [H, I], bf16, kind="ExternalInput").ap()
    Wd = nc.dram_tensor("Wd", [I, H], bf16, kind="ExternalInput").ap()
    yT = nc.dram_tensor("yT", [H, C], fp32, kind="ExternalOutput").ap()

    segs = _tsegs(C)
    NSEG = len(segs)

    with tile.TileContext(nc) as tc, \
         tc.tile_pool(name="wp", bufs=1) as wp, \
         tc.tile_pool(name="xp", bufs=1) as xp, \
         tc.tile_pool(name="hp", bufs=2) as hp, \
         tc.tile_pool(name="evA", bufs=2) as evA, \
         tc.tile_pool(name="evB", bufs=4) as evB, \
         tc.tile_pool(name="psA", bufs=2, space="PSUM") as psA, \
         tc.tile_pool(name="psB", bufs=1, space="PSUM") as psB:

        wg_s = [[None] * NWC for _ in range(KB)]
        wu_s = [[None] * NWC for _ in range(KB)]
        xt_s = [[None] * NSEG for _ in range(KB)]

        # --- DMA issue order (sync queue, FIFO): startup-critical first. ---
        # First A i-block needs wg[k][0], wu[k][0], xt[k][seg0] for all k.
        t0_0, tb_0 = segs[0]
        for k in range(KB):
            wgk = wp.tile([PB, WCOL], bf16, name=f"wg{k}_0")
            nc.sync.dma_start(out=wgk[:], in_=Wg[k * PB:(k + 1) * PB, 0:WCOL])
            wg_s[k][0] = wgk
            wuk = wp.tile([PB, WCOL], bf16, name=f"wu{k}_0")
            nc.sync.dma_start(out=wuk[:], in_=Wu[k * PB:(k + 1) * PB, 0:WCOL])
            wu_s[k][0] = wuk
            xtk = xp.tile([PB, tb_0], bf16, name=f"xt{k}_0")
            nc.sync.dma_start(
                out=xtk[:], in_=xT[k * PB:(k + 1) * PB, t0_0:t0_0 + tb_0])
            xt_s[k][0] = xtk
        # Remaining Wg/Wu column tiles in i-block need order.
        for c in range(1, NWC):
            for k in range(KB):
                wgk = wp.tile([PB, WCOL], bf16, name=f"wg{k}_{c}")
                nc.sync.dma_start(
                    out=wgk[:],
                    in_=Wg[k * PB:(k + 1) * PB, c * WCOL:(c + 1) * WCOL])
                wg_s[k][c] = wgk
            for k in range(KB):
                wuk = wp.tile([PB, WCOL], bf16, name=f"wu{k}_{c}")
                nc.sync.dma_start(
                    out=wuk[:],
                    in_=Wu[k * PB:(k + 1) * PB, c * WCOL:(c + 1) * WCOL])
                wu_s[k][c] = wuk
        # Second token segment, then Wd (needed when B of seg0 starts),
        # gates, then the remaining token segments.
        if NSEG > 1:
            t0_1, tb_1 = segs[1]
            for k in range(KB):
                xtk = xp.tile([PB, tb_1], bf16, name=f"xt{k}_1")
                nc.sync.dma_start(
                    out=xtk[:], in_=xT[k * PB:(k + 1) * PB, t0_1:t0_1 + tb_1])
                xt_s[k][1] = xtk
        wd_s = []
        for ib in range(IB):
            wdk = wp.tile([PB, H], bf16, name=f"wd{ib}")
            nc.sync.dma_start(out=wdk[:], in_=Wd[ib * PB:(ib + 1) * PB, :])
            wd_s.append(wdk)
        gt = wp.tile([PB, C], fp32, name="gt")
        nc.sync.dma_start(out=gt[:], in_=gm[:])
        for si in range(2, NSEG):
            t0s, tbs = segs[si]
            for k in range(KB):
                xtk = xp.tile([PB, tbs], bf16, name=f"xt{k}_{si}")
                nc.sync.dma_start(
                    out=xtk[:], in_=xT[k * PB:(k + 1) * PB, t0s:t0s + tbs])
                xt_s[k][si] = xtk

        # --- Compute: per segment, A (h into SBUF) then B (two half passes).
        for si, (t0, tb) in enumerate(segs):
            h_tiles = []
            for ib in range(IB):
                c, j = ib // 2, (ib % 2) * PB
                pg = psA.tile([PB, tb], fp32, tag="pg", name="pg")
                pu = psA.tile([PB, tb], fp32, tag="pu", name="pu")
                for k in range(KB):
                    nc.tensor.matmul(
                        pg[:], wg_s[k][c][:, j:j + PB], xt_s[k][si][:],
                        start=(k == 0), stop=(k == KB - 1))
                for k in range(KB):
                    nc.tensor.matmul(
                        pu[:], wu_s[k][c][:, j:j + PB], xt_s[k][si][:],
                        start=(k == 0), stop=(k == KB - 1))
                sg = evA.tile([PB, tb], fp32, tag="sg", name="sg")
                nc.scalar.activation(sg[:], pg[:], AF.Silu)
                hh = hp.tile([PB, tb], bf16, tag=f"h{ib}", name=f"h{ib}")
                nc.vector.tensor_mul(hh[:], sg[:], pu[:])
                h_tiles.append(hh)
            for half in range(2):
                pys = [psB.tile([PB, tb], fp32, tag=f"py{j}", name=f"py{j}")
                       for j in range(HB // 2)]
                for ib in range(IB):
                    last = ib == IB - 1
                    for j in range(HB // 2):
                        hb = half * (HB // 2) + j
                        nc.tensor.matmul(
                            pys[j][:],
                            wd_s[ib][:, hb * PB:(hb + 1) * PB],
                            h_tiles[ib][:],
                            start=(ib == 0), stop=last)
                        if last:
                            yt = evB.tile([PB, tb], fp32, tag="yt", name="yt")
                            nc.vector.tensor_mul(yt[:], pys[j][:],
                                                 gt[:, t0:t0 + tb])
                            eng = (nc.sync if si == NSEG - 1 else nc.gpsimd)
                            eng.dma_start(
                                out=yT[hb * PB:(hb + 1) * PB, t0:t0 + tb],
                                in_=yt[:])
    nc.compile()
    return nc


def _route(x, Wr, br):
    """Replicate the reference's fp32 router bit-compatibly on host."""
    logits = x @ Wr + br                       # fp32 GEMM
    order = np.argsort(-logits, axis=1, kind="stable")  # ties -> lowest index
    topk_idx = order[:, :TOPK]
    topk_vals = np.take_along_axis(logits, topk_idx, axis=1)
    g = 1.0 / (1.0 + np.exp(-topk_vals.astype(np.float32)))
    g = g / (np.sum(g, axis=-1, keepdims=True) + 1e-10)
    return topk_idx, g.astype(np.float32)


def kernel(x, Wr, br, Wg, Wu, Wd):
    global last_results
    from concourse.bass_utils import run_bass_kernel_spmd

    x = np.asarray(x, dtype=np.float32)
    Wr = np.asarray(Wr, dtype=np.float32)
    br = np.asarray(br, dtype=np.float32)
    Wg = np.asarray(Wg, dtype=np.float32)
    Wu = np.asarray(Wu, dtype=np.float32)
    Wd = np.asarray(Wd, dtype=np.float32)

    topk_idx, g = _route(x, Wr, br)

    # Per-expert token lists
    idx_lists = []
    gate_lists = []
    for e in range(E):
        mask = topk_idx == e                    # [T, K]
        tok = np.nonzero(mask.any(axis=1))[0]
        gsel = np.where(mask[tok, 0], g[tok, 0], g[tok, 1]).astype(np.float32)
        idx_lists.append(tok.astype(np.int64))
        gate_lists.append(gsel)

    counts = [len(ix) for ix in idx_lists]
    C = max(512, -(-max(counts) // 8) * 8)     # pad to a multiple of 8

    if C not in _compiled:
        _compiled[C] = _build(C)
    nc = _compiled[C]

    bf = ml_dtypes.bfloat16
    xTf = np.ascontiguousarray(x.T).astype(bf)   # [H, T] bf16
    in_maps = []
    for e in range(E):
        n = counts[e]
        xTe = np.zeros((H, C), dtype=bf)
        xTe[:, :n] = xTf[:, idx_lists[e]]
        gme = np.zeros((PB, C), dtype=np.float32)
        gme[:, :n] = gate_lists[e][None, :]
        in_maps.append({
            "xT": xTe,
            "gm": gme,
            "Wg": Wg[e].astype(bf),
            "Wu": Wu[e].astype(bf),
            "Wd": Wd[e].astype(bf),
        })

    trace = bool(int(os.environ.get("MOE_TRACE", "0")))
    trace_cores = (list(range(NCORES))
                   if os.environ.get("MOE_TRACE_ALL") else None)
    last_results = run_bass_kernel_spmd(
        nc, in_maps, core_ids=list(range(NCORES)), trace=trace,
        trace_cores=trace_cores)

    out = np.zeros((T, H), dtype=np.float32)
    for e in range(E):
        n = counts[e]
        yTe = last_results.results[e]["yT"]
        out[idx_lists[e]] += yTe[:, :n].T
    return out


# revision 3
# speedup vs baseline: 1.1158x; 1.0854x over previous
"""MoE (top-2 of 8 experts, SwiGLU) Trainium2 kernel.

Strategy (expert-parallel over 8 NeuronCores):
  * Host: router GEMM + top-2 + sigmoid gates in numpy (selection matches the
    jax fp32 reference on these inputs), then gather each expert's tokens into
    a transposed, capacity-padded bf16 buffer xT_e [H, C]. One expert per core.
  * Device (SPMD, per core): fully fused single pass in bf16.
    All weights (Wg, Wu, Wd ~ 12.6 MB bf16) stay SBUF-resident; the h
    intermediate never leaves SBUF (no DRAM spill).  Per 512-token segment:
      A: for each of 16 I-blocks: pg = x@Wg, pu = x@Wu (PSUM, 8 k-matmuls
         each), then h = silu(pg)*pu via Scalar(Silu) + one DVE mul with a
         bf16 output into SBUF.
      B: yT = (h @ Wd) * gate in two half-passes of 4 output blocks
         (4 PSUM banks each; A double-buffers the other 4), gates applied
         during PSUM eviction into a wide staging tile, one store per pass.
    bf16 matters twice: LDWEIGHTS of a 128x128 bf16 stationary (~0.1us with
    FWL) hides fully under the 213ns/512-row matmul stream, whereas fp32r
    stationary loads (~224ns) gate the pipe; and all weight/x DMA halves.
  * DMA: loads are batched with 3D access patterns (one descriptor-generation
    instruction per 256-column weight slice / per token segment) because each
    dma_start costs ~0.7us of sequencer issue time; ~25 loads total.
  * HAM warmup: ~26 dummy matmuls on a zeroed scratch tile run during the
    startup DMA so the PE clock (1.2 GHz cold -> 2.4 GHz after ~3.4us busy)
    is warm when the real matmuls start.
  * Host: out[idx_e] += yT_e[:, :n_e].T  (indices within one expert are
    unique, so fancy-index += is safe).
"""

import os
import numpy as np
import ml_dtypes

T, H, I, E, TOPK = 8192, 1024, 2048, 8, 2
NCORES = 8
PB = 128
KB = H // PB     # 8 contraction blocks over H
IB = I // PB     # 16 blocks over I
HB = H // PB     # 8 output blocks over H
WCOL = 256       # Wg/Wu load-slice column width (2 I-blocks per slice)
NWC = I // WCOL  # 8 column slices
NWARM = 26       # HAM warmup matmuls

_compiled = {}
last_results = None  # BassKernelResults of the most recent run (for test harness)


def _tsegs(C):
    """Split C into 512-wide segments plus a remainder segment."""
    segs = []
    t0 = 0
    while C - t0 >= 512:
        segs.append((t0, 512))
        t0 += 512
    if C - t0:
        segs.append((t0, C - t0))
    return segs


def _build(C):
    import concourse.bacc as bacc
    import concourse.mybir as mybir
    import concourse.tile as tile

    fp32 = mybir.dt.float32
    bf16 = mybir.dt.bfloat16
    AF = mybir.ActivationFunctionType

    nc = bacc.Bacc("TRN2", target_bir_lowering=False, debug=False,
                   num_devices=NCORES)
    xT = nc.dram_tensor("xT", [H, C], bf16, kind="ExternalInput").ap()
    gm = nc.dram_tensor("gm", [PB, C], fp32, kind="ExternalInput").ap()
    Wg = nc.dram_tensor("Wg", [H, I], bf16, kind="ExternalInput").ap()
    Wu = nc.dram_tensor("Wu", [H, I], bf16, kind="ExternalInput").ap()
    Wd = nc.dram_tensor("Wd", [I, H], bf16, kind="ExternalInput").ap()
    yT = nc.dram_tensor("yT", [H, C], fp32, kind="ExternalOutput").ap()

    # Batched-DMA views: partition-major with the k/ib block index as a free
    # axis, so one dma_start moves all 8 (16) row-blocks of a column slice.
    Wg3 = Wg.rearrange("(k p) j -> p k j", k=KB)     # [128, 8, 2048]
    Wu3 = Wu.rearrange("(k p) j -> p k j", k=KB)
    Wd3 = Wd.rearrange("(b p) j -> p b j", b=IB)     # [128, 16, 1024]
    xT3 = xT.rearrange("(k p) t -> p k t", k=KB)     # [128, 8, C]
    yT3 = yT.rearrange("(b p) t -> p b t", b=HB)     # [128, 8, C]

    segs = _tsegs(C)
    NSEG = len(segs)
    HH = HB // 2

    with tile.TileContext(nc) as tc, \
         tc.tile_pool(name="wp", bufs=1) as wp, \
         tc.tile_pool(name="xp", bufs=1) as xp, \
         tc.tile_pool(name="hp", bufs=2) as hp, \
         tc.tile_pool(name="evA", bufs=2) as evA, \
         tc.tile_pool(name="evB", bufs=2) as evB, \
         tc.tile_pool(name="psA", bufs=2, space="PSUM") as psA, \
         tc.tile_pool(name="psB", bufs=1, space="PSUM") as psB:

        # --- HAM warmup: dummy matmuls on a zeroed scratch tile. ---
        sc = wp.tile([PB, 512], bf16, name="scwarm")
        nc.vector.memset(sc[:], 0)
        pw = psA.tile([PB, 512], fp32, tag="pg", name="pgw")
        for _ in range(NWARM):
            nc.tensor.matmul(pw[:], sc[:, 0:PB], sc[:], start=True, stop=True)

        # --- Loads (sync queue, FIFO), startup-critical first. ---
        t0_0, tb_0 = segs[0]
        xt_s = [None] * NSEG
        xt0 = xp.tile([PB, KB * tb_0], bf16, name="xt0")
        nc.sync.dma_start(
            out=xt0[:].rearrange("p (k t) -> p k t", k=KB),
            in_=xT3[:, :, t0_0:t0_0 + tb_0])
        xt_s[0] = xt0
        wg_s = [None] * NWC
        wu_s = [None] * NWC
        for c in range(NWC):
            wgc = wp.tile([PB, KB * WCOL], bf16, name=f"wg{c}")
            nc.sync.dma_start(
                out=wgc[:].rearrange("p (k j) -> p k j", k=KB),
                in_=Wg3[:, :, c * WCOL:(c + 1) * WCOL])
            wg_s[c] = wgc
            wuc = wp.tile([PB, KB * WCOL], bf16, name=f"wu{c}")
            nc.sync.dma_start(
                out=wuc[:].rearrange("p (k j) -> p k j", k=KB),
                in_=Wu3[:, :, c * WCOL:(c + 1) * WCOL])
            wu_s[c] = wuc
        if NSEG > 1:
            t0_1, tb_1 = segs[1]
            xt1 = xp.tile([PB, KB * tb_1], bf16, name="xt1")
            nc.sync.dma_start(
                out=xt1[:].rearrange("p (k t) -> p k t", k=KB),
                in_=xT3[:, :, t0_1:t0_1 + tb_1])
            xt_s[1] = xt1
        wd = wp.tile([PB, IB * H], bf16, name="wd")
        nc.sync.dma_start(
            out=wd[:].rearrange("p (b j) -> p b j", b=IB),
            in_=Wd3[:, :, :])
        gt = wp.tile([PB, C], fp32, name="gt")
        nc.sync.dma_start(out=gt[:], in_=gm[:])
        for si in range(2, NSEG):
            t0s, tbs = segs[si]
            xts = xp.tile([PB, KB * tbs], bf16, name=f"xt{si}")
            nc.sync.dma_start(
                out=xts[:].rearrange("p (k t) -> p k t", k=KB),
                in_=xT3[:, :, t0s:t0s + tbs])
            xt_s[si] = xts

        # --- Compute: per segment, A (h into SBUF) then B (two half passes).
        for si, (t0, tb) in enumerate(segs):
            xts = xt_s[si]
            h_tiles = []
            for ib in range(IB):
                c, j = ib // 2, (ib % 2) * PB
                pg = psA.tile([PB, tb], fp32, tag="pg", name="pg")
                pu = psA.tile([PB, tb], fp32, tag="pu", name="pu")
                for k in range(KB):
                    nc.tensor.matmul(
                        pg[:], wg_s[c][:, k * WCOL + j:k * WCOL + j + PB],
                        xts[:, k * tb:(k + 1) * tb],
                        start=(k == 0), stop=(k == KB - 1))
                for k in range(KB):
                    nc.tensor.matmul(
                        pu[:], wu_s[c][:, k * WCOL + j:k * WCOL + j + PB],
                        xts[:, k * tb:(k + 1) * tb],
                        start=(k == 0), stop=(k == KB - 1))
                sg = evA.tile([PB, tb], fp32, tag="sg", name="sg")
                nc.scalar.activation(sg[:], pg[:], AF.Silu)
                hh = hp.tile([PB, tb], bf16, tag=f"h{ib}", name=f"h{ib}")
                nc.vector.tensor_mul(hh[:], sg[:], pu[:])
                h_tiles.append(hh)
            for half in range(2):
                pys = [psB.tile([PB, tb], fp32, tag=f"py{j}", name=f"py{j}")
                       for j in range(HH)]
                ytp = evB.tile([PB, HH * tb], fp32, tag="yt", name="yt")
                for ib in range(IB):
                    last = ib == IB - 1
                    for j in range(HH):
                        hb = half * HH + j
                        nc.tensor.matmul(
                            pys[j][:],
                            wd[:, ib * H + hb * PB:ib * H + hb * PB + PB],
                            h_tiles[ib][:],
                            start=(ib == 0), stop=last)
                        if last:
                            nc.vector.tensor_mul(
                                ytp[:, j * tb:(j + 1) * tb], pys[j][:],
                                gt[:, t0:t0 + tb])
                eng = (nc.sync if si == NSEG - 1 else nc.gpsimd)
                eng.dma_start(
                    out=yT3[:, half * HH:(half + 1) * HH, t0:t0 + tb],
                    in_=ytp[:].rearrange("p (b t) -> p b t", b=HH))
    nc.compile()
    return nc


def _route(x, Wr, br):
    """Replicate the reference's fp32 router bit-compatibly on host."""
    logits = x @ Wr + br                       # fp32 GEMM
    order = np.argsort(-logits, axis=1, kind="stable")  # ties -> lowest index
    topk_idx = order[:, :TOPK]
    topk_vals = np.take_along_axis(logits, topk_idx, axis=1)
    g = 1.0 / (1.0 + np.exp(-topk_vals.astype(np.float32)))
    g = g / (np.sum(g, axis=-1, keepdims=True) + 1e-10)
    return topk_idx, g.astype(np.float32)


def kernel(x, Wr, br, Wg, Wu, Wd):
    global last_results
    from concourse.bass_utils import run_bass_kernel_spmd

    x = np.asarray(x, dtype=np.float32)
    Wr = np.asarray(Wr, dtype=np.float32)
    br = np.asarray(br, dtype=np.float32)
    Wg = np.asarray(Wg, dtype=np.float32)
    Wu = np.asarray(Wu, dtype=np.float32)
    Wd = np.asarray(Wd, dtype=np.float32)

    topk_idx, g = _route(x, Wr, br)

    # Per-expert token lists
    idx_lists = []
    gate_lists = []
    for e in range(E):
        mask = topk_idx == e                    # [T, K]
        tok = np.nonzero(mask.any(axis=1))[0]
        gsel = np.where(mask[tok, 0], g[tok, 0], g[tok, 1]).astype(np.float32)
        idx_lists.append(tok.astype(np.int64))
        gate_lists.append(gsel)

    counts = [len(ix) for ix in idx_lists]
    C = max(512, -(-max(counts) // 8) * 8)     # pad to a multiple of 8

    if C not in _compiled:
        _compiled[C] = _build(C)
    nc = _compiled[C]

    bf = ml_dtypes.bfloat16
    xTf = np.ascontiguousarray(x.T).astype(bf)   # [H, T] bf16
    in_maps = []
    for e in range(E):
        n = counts[e]
        xTe = np.zeros((H, C), dtype=bf)
        xTe[:, :n] = xTf[:, idx_lists[e]]
        gme = np.zeros((PB, C), dtype=np.float32)
        gme[:, :n] = gate_lists[e][None, :]
        in_maps.append({
            "xT": xTe,
            "gm": gme,
            "Wg": Wg[e].astype(bf),
            "Wu": Wu[e].astype(bf),
            "Wd": Wd[e].astype(bf),
        })

    trace = bool(int(os.environ.get("MOE_TRACE", "0")))
    trace_cores = (list(range(NCORES))
                   if os.environ.get("MOE_TRACE_ALL") else None)
    last_results = run_bass_kernel_spmd(
        nc, in_maps, core_ids=list(range(NCORES)), trace=trace,
        trace_cores=trace_cores)

    out = np.zeros((T, H), dtype=np.float32)
    for e in range(E):
        n = counts[e]
        yTe = last_results.results[e]["yT"]
        out[idx_lists[e]] += yTe[:, :n].T
    return out
